# revision 1
# baseline (speedup 1.0000x reference)
"""Distributed GQA attention layer (dense_transformer) on 8 TRN2 NeuronCores.

Sharding: 8-way tensor parallel over heads. Core c owns q-heads [4c..4c+4),
kv-head c, and the matching 512 columns/rows of Wq/Wk/Wv/Wo. Each core
computes its heads' attention for both batch rows, the per-core context is
AllGathered (bf16, 4MB/rank), and each core produces a disjoint 512-wide
slice of the output hidden dim via its Wo shard. Host assembles by pure
concatenation.

Layout strategy (per core):
  - hidden^T (bf16, host-pretransposed) streams through SBUF once.
  - QKV projections produce q^T/k^T/v^T [dim, token] directly (weight-
    stationary matmuls, N=512 moving).
  - RoPE applied in [dim, token] layout: partition-swap via a permutation
    matmul on PE, then q*cos + swap*sin on DVE with host-precomputed
    [128, S] tables (sign folded into the sin table, softmax scale folded
    into Wq).
  - Scores are computed transposed: scores^T[s_k, s_q] = k^T.T @ q^T, so
    softmax exp tiles feed PV directly as the moving operand:
    ctx^T[d, s_q] = V[s_k, d].T @ exp[s_k, s_q], with the denominator from
    a parallel ones-vector matmul. Causal masking = skip fully-masked
    chunks + one triangular 128x128 mask on diagonal blocks.
  - o_proj contracts over the gathered [4096, token] context with the Wo
    shard SBUF-resident.
  - The AllGather is split into 8 per-token-block gathers (512KB/rank each)
    and block o_proj is emitted after the NEXT block's attention, so both
    the collectives and the gathered-context DMAs hide under attention
    compute on the in-order PE stream.
"""
import sys
sys.path.insert(0, "/opt/trn_rl_repo")

import numpy as np
import ml_dtypes

import concourse.bass as bass
import concourse.tile as tile
from concourse import bacc, mybir

BF16 = mybir.dt.bfloat16
F32 = mybir.dt.float32
NPBF16 = ml_dtypes.bfloat16

N_CORES = 8
B, S, HID = 2, 2048, 4096
NH, KVH, D = 32, 8, 128
TOK = B * S                # 4096 tokens, batch-major
QO = NH * D // N_CORES     # 512 q-out dims per core
TT = 512                   # token tile (moving free dim)
NTT = TOK // TT            # 8 token tiles
KC = HID // 128            # 32 contraction chunks


def _build(sim=False, loop_k=1, simpden=False):
    # sim=True: single-core variant for TimelineSim (cost-model timing) —
    # the AllGather is replaced by a local DMA of this core's slice.
    # loop_k>1: timing variant — each compute phase repeats loop_k times
    # inside a hardware For_i loop so device time dominates dispatch noise.
    nc = bacc.Bacc("TRN2", target_bir_lowering=False, debug=False,
                   num_devices=1 if sim else N_CORES)
    import contextlib

    if isinstance(loop_k, int):
        loop_k = (loop_k, loop_k, loop_k)
    _phase_idx = iter([0, 1, 2])

    def phase_loop(tc):
        k = loop_k[next(_phase_idx)]
        if k > 1:
            return tc.For_i(0, k, 1)
        return contextlib.nullcontext()
    hid_t = nc.dram_tensor("hid_t", [HID, TOK], BF16, kind="ExternalInput").ap()
    wq_t = nc.dram_tensor("wq_t", [HID, QO], BF16, kind="ExternalInput").ap()
    wk_t = nc.dram_tensor("wk_t", [HID, D], BF16, kind="ExternalInput").ap()
    wv_t = nc.dram_tensor("wv_t", [HID, D], BF16, kind="ExternalInput").ap()
    wo_t = nc.dram_tensor("wo_t", [HID, QO], BF16, kind="ExternalInput").ap()
    cos_t = nc.dram_tensor("cos_t", [D, S], F32, kind="ExternalInput").ap()
    sin_t = nc.dram_tensor("sin_t", [D, S], F32, kind="ExternalInput").ap()
    perm_d = nc.dram_tensor("perm", [128, 128], BF16, kind="ExternalInput").ap()
    ident_d = nc.dram_tensor("ident", [128, 128], BF16, kind="ExternalInput").ap()
    tri_d = nc.dram_tensor("tri", [128, 128], BF16, kind="ExternalInput").ap()
    ones4_d = nc.dram_tensor("ones4", [128, 128], BF16, kind="ExternalInput").ap()
    out = nc.dram_tensor("out", [TOK, QO], F32, kind="ExternalOutput").ap()

    EXP = mybir.ActivationFunctionType.Exp

    with tile.TileContext(nc) as tc:
        with tc.tile_pool(name="const", bufs=1) as cst, \
             tc.tile_pool(name="persist", bufs=1) as per, \
             tc.tile_pool(name="dram", bufs=1, space="DRAM") as dram:
            cos_sb = cst.tile([D, S], F32)
            nc.sync.dma_start(out=cos_sb, in_=cos_t)
            sin_sb = cst.tile([D, S], F32)
            nc.sync.dma_start(out=sin_sb, in_=sin_t)
            perm_sb = cst.tile([128, 128], BF16)
            nc.sync.dma_start(out=perm_sb, in_=perm_d)
            ident_sb = cst.tile([128, 128], BF16)
            nc.sync.dma_start(out=ident_sb, in_=ident_d)
            tri_sb = cst.tile([128, 128], BF16)
            nc.sync.dma_start(out=tri_sb, in_=tri_d)
            ones_sb = cst.tile([128, 1], BF16)
            nc.vector.memset(ones_sb, 1.0)
            ones4_sb = cst.tile([128, 128], BF16)
            nc.sync.dma_start(out=ones4_sb, in_=ones4_d)

            q_rope = per.tile([128, 4, TOK], BF16)    # [d, head, token]
            k_rope = per.tile([128, TOK], BF16)       # [d, token]
            v_sb = per.tile([128, KC, 128], BF16)     # [tok%128, tokchunk, d]

            cc_in = [dram.tile([QO, TT], BF16, name=f"ccin{i}")
                     for i in range(NTT)]
            cc_out = [dram.tile([N_CORES * QO, TT], BF16, addr_space="Shared",
                                name=f"ccout{i}")
                      for i in range(NTT)]

            # ---------------- QKV projections + RoPE ----------------
            with tc.tile_pool(name="wqkv", bufs=1) as wp, \
                 tc.tile_pool(name="hin", bufs=2) as hp, \
                 tc.tile_pool(name="qk_ps", bufs=1, space="PSUM") as aps, \
                 tc.tile_pool(name="rope_ps", bufs=1, space="PSUM") as rps, \
                 tc.tile_pool(name="ropesb", bufs=2) as rsb:
                wq_sb = wp.tile([128, KC, QO], BF16)
                wq_r = wq_t.rearrange("(c p) m -> p c m", p=128)
                for q4 in range(4):
                    eng = nc.sync if q4 % 2 == 0 else nc.scalar
                    eng.dma_start(out=wq_sb[:, q4 * 8:(q4 + 1) * 8, :],
                                  in_=wq_r[:, q4 * 8:(q4 + 1) * 8, :])
                wk_sb = wp.tile([128, KC, D], BF16)
                nc.sync.dma_start(out=wk_sb,
                                  in_=wk_t.rearrange("(c p) m -> p c m", p=128))
                wv_sb = wp.tile([128, KC, D], BF16)
                nc.scalar.dma_start(out=wv_sb,
                                    in_=wv_t.rearrange("(c p) m -> p c m", p=128))

                hid_r = hid_t.rearrange("(c p) t -> p c t", p=128)
                with phase_loop(tc):
                  for tt in range(NTT):
                      pos0 = (tt % (S // TT)) * TT
                      h_tile = hp.tile([128, KC, TT], BF16, tag="h")
                      for q4 in range(4):
                          eng = nc.sync if q4 % 2 == 0 else nc.gpsimd
                          eng.dma_start(
                              out=h_tile[:, q4 * 8:(q4 + 1) * 8, :],
                              in_=hid_r[:, q4 * 8:(q4 + 1) * 8,
                                        tt * TT:(tt + 1) * TT])

                      accs = [aps.tile([128, TT], F32, tag=f"acc{i}",
                                       name=f"acc{i}")
                              for i in range(6)]
                      for kc in range(KC):
                          st, sp = kc == 0, kc == KC - 1
                          rhs = h_tile[:, kc, :]
                          for m in range(4):
                              nc.tensor.matmul(
                                  accs[m], lhsT=wq_sb[:, kc, m * 128:(m + 1) * 128],
                                  rhs=rhs, start=st, stop=sp)
                          nc.tensor.matmul(accs[4], lhsT=wk_sb[:, kc, :], rhs=rhs,
                                           start=st, stop=sp)
                          nc.tensor.matmul(accs[5], lhsT=wv_sb[:, kc, :], rhs=rhs,
                                           start=st, stop=sp)

                      # RoPE for the 4 q chunks + 1 k chunk
                      cs = cos_sb[:, pos0:pos0 + TT]
                      ss = sin_sb[:, pos0:pos0 + TT]
                      for m in range(5):
                          acc = accs[m]
                          xbf = rsb.tile([128, TT], BF16, tag="xbf")
                          nc.scalar.copy(xbf, acc)
                          swp = rps.tile([128, TT], F32, tag="swp")
                          nc.tensor.matmul(swp, lhsT=perm_sb, rhs=xbf,
                                           start=True, stop=True)
                          t2 = rsb.tile([128, TT], F32, tag="t2")
                          nc.vector.tensor_mul(t2, swp, ss)
                          t1 = rsb.tile([128, TT], F32, tag="t1")
                          nc.vector.tensor_mul(t1, acc, cs)
                          if m < 4:
                              dest = q_rope[:, m, tt * TT:(tt + 1) * TT]
                          else:
                              dest = k_rope[:, tt * TT:(tt + 1) * TT]
                          nc.vector.tensor_add(dest, t1, t2)

                      # V: cast + transpose chunks into [token, d] layout
                      vbf = rsb.tile([128, TT], BF16, tag="vbf")
                      nc.scalar.copy(vbf, accs[5])
                      for j in range(4):
                          vtp = rps.tile([128, 128], BF16, tag="vtp")
                          nc.tensor.transpose(vtp, vbf[:, j * 128:(j + 1) * 128],
                                              ident_sb)
                          nc.vector.tensor_copy(v_sb[:, tt * 4 + j, :], vtp)

            # ------------- attention / AllGather / o_proj (fused) -------------
            # Block pipeline over NTT token blocks of 512: attention for the
            # block's 4 heads -> per-block AllGather -> block o_proj, with
            # o_proj(blk) emitted after attention(blk+1) so the PE never
            # waits on a collective in flight.
            def emit_attn_tile(cps, asb, b, h, t, cc_dst, sc_bufs):
                tok0 = b * S + t * TT
                nkc = 4 * t + 4
                ctx = cps.tile([128, TT], F32, tag="ctx", bufs=2, name="ctx")
                # den rows {0,32,64,96} hold 4 partial denominators from
                # col-tiled ones-matmuls (they execute concurrently when
                # adjacent in the PE stream); other rows stay zero from the
                # one-time phase-start memset.
                den = cps.tile([128, TT], F32, tag="den", bufs=1, name="den")
                if t == 0:
                    # chunks 1..3 first write only cols [a0:], so clear the
                    # stale prefixes left by the previous tile in this bank
                    for cg in range(1, 4):
                        nc.vector.memset(den[cg * 32:cg * 32 + 1,
                                             0:cg * 128], 0.0)

                def a0_of(kc):
                    return max(kc * 128 - t * TT, 0)

                # software pipeline: PE stream is score(k+2), pv(k), with
                # den matmuls batched in adjacent groups of 4 (one per PE
                # column group) so they run concurrently.
                scs, exs = {}, {}
                for kc in range(nkc + 2):
                    if kc < nkc:
                        a0 = a0_of(kc)
                        sc = cps.tile([128, TT], F32, tag="sc",
                                      bufs=sc_bufs, name="sc")
                        nc.tensor.matmul(
                            sc[:, a0:],
                            lhsT=k_rope[:, b * S + kc * 128:
                                        b * S + (kc + 1) * 128],
                            rhs=q_rope[:, h, tok0 + a0:tok0 + TT],
                            start=True, stop=True)
                        scs[kc] = sc
                    if 1 <= kc <= nkc:
                        j = kc - 1
                        a0 = a0_of(j)
                        sc = scs.pop(j)
                        ex = asb.tile([128, TT], BF16, tag="ex",
                                      bufs=6, name="ex")
                        nc.scalar.activation(ex[:, a0:], sc[:, a0:], EXP)
                        if a0 == j * 128 - t * TT:
                            # diagonal block: triangular mask
                            nc.vector.tensor_mul(ex[:, a0:a0 + 128],
                                                 ex[:, a0:a0 + 128], tri_sb)
                        exs[j] = ex
                    if kc >= 2:
                        j = kc - 2
                        a0 = a0_of(j)
                        st, sp = j == 0, j == nkc - 1
                        nc.tensor.matmul(ctx[:, a0:],
                                         lhsT=v_sb[:, b * 16 + j, :],
                                         rhs=exs[j][:, a0:],
                                         start=st, stop=sp)
                        if simpden:
                            nc.tensor.matmul(den[0:1, a0:], lhsT=ones_sb,
                                             rhs=exs.pop(j)[:, a0:],
                                             start=st, stop=sp)
                        elif j % 4 == 3:
                            for jj in range(j - 3, j + 1):
                                cg = jj % 4
                                aj = a0_of(jj)
                                nc.tensor.matmul(
                                    den[cg * 32:cg * 32 + 1, aj:],
                                    lhsT=ones_sb, rhs=exs[jj][:, aj:],
                                    start=jj < 4, stop=jj >= nkc - 4,
                                    tile_position=(0, cg * 32))
                                exs.pop(jj)
                if simpden:
                    rd1 = asb.tile([1, TT], F32, tag="rd1")
                    nc.vector.reciprocal(rd1, den[0:1, :])
                    rden = asb.tile([128, TT], F32, tag="rden")
                    nc.gpsimd.partition_broadcast(rden, rd1)
                else:
                    # sum the 4 partial denominator rows and broadcast to
                    # all 128 partitions in one matmul with the ones4
                    # selector
                    dencp = asb.tile([128, TT], BF16, tag="dencp")
                    nc.vector.tensor_copy(dencp, den)
                    # reuse the den bank: it frees exactly when dencp is
                    # copied, which is this matmul's input dependency anyway
                    bcast = cps.tile([128, TT], F32, tag="den", bufs=1,
                                     name="bcast")
                    nc.tensor.matmul(bcast, lhsT=ones4_sb, rhs=dencp,
                                     start=True, stop=True)
                    rden = asb.tile([128, TT], F32, tag="rden")
                    nc.vector.reciprocal(rden, bcast)
                ctxn = asb.tile([128, TT], BF16, tag="ctxn")
                nc.vector.tensor_mul(ctxn, ctx, rden)
                nc.sync.dma_start(out=cc_dst, in_=ctxn)

            def emit_oproj_mg(ops, osb, wo_sb, mg, src_r, src_c0):
                # two passes of 2 output m-tiles each: same matmul count,
                # half the PSUM banks (leaves room for the attention's
                # denominator machinery)
                g = osb.tile([128, KC, TT], BF16, tag="g", bufs=2)
                # split the 4MB load across both HWDGE rings
                for q4 in range(4):
                    eng = nc.sync if q4 % 2 == 0 else nc.scalar
                    eng.dma_start(
                        out=g[:, q4 * 8:(q4 + 1) * 8, :],
                        in_=src_r[:, q4 * 8:(q4 + 1) * 8,
                                  src_c0:src_c0 + TT])
                for m in range(4):
                    # 32 consecutive same-bank accumulations per output
                    # tile (measured equal to bank-alternating 2x2 passes)
                    omt = ops.tile([128, QO], F32, tag="om", bufs=3,
                                   name="omt")
                    for kc in range(KC):
                        nc.tensor.matmul(
                            omt,
                            lhsT=g[:, kc, m * 128:(m + 1) * 128],
                            rhs=wo_sb[:, kc, :],
                            start=kc == 0, stop=kc == KC - 1)
                    ofin = osb.tile([128, QO], F32, tag="ofin", bufs=2)
                    nc.vector.tensor_copy(ofin, omt)
                    nc.gpsimd.dma_start(
                        out=out[mg * TT + m * 128:
                                mg * TT + (m + 1) * 128, :],
                        in_=ofin)

            no_collective = sim or (loop_k[1] > 1)
            with tc.tile_pool(name="fu_ps", bufs=1, space="PSUM") as cps, \
                 tc.tile_pool(name="at_sb", bufs=2) as asb, \
                 tc.tile_pool(name="wo", bufs=1) as wop, \
                 tc.tile_pool(name="o_ps", bufs=1, space="PSUM") as ops, \
                 tc.tile_pool(name="o_sb", bufs=3) as osb:
                wo_sb = wop.tile([128, KC, QO], BF16)
                nc.sync.dma_start(out=wo_sb,
                                  in_=wo_t.rearrange("(c p) m -> p c m", p=128))
                blk_out_r = [
                    cc_out[blk][:].rearrange("(c p) t -> p c t", p=128)
                    for blk in range(NTT)
                ]
                den_init = cps.tile([128, TT], F32, tag="den",
                                    name="den_init")
                nc.vector.memset(den_init, 0.0)
                with phase_loop(tc):
                  for blk in range(NTT + 1):
                    if blk < NTT:
                        b, t = divmod(blk, S // TT)
                        for h in range(4):
                            emit_attn_tile(
                                cps, asb, b, h, t,
                                cc_in[blk][h * 128:(h + 1) * 128, :],
                                sc_bufs=2)
                        if no_collective:
                            nc.sync.dma_start(out=cc_out[blk][0:QO, :],
                                              in_=cc_in[blk][:])
                        else:
                            nc.gpsimd.collective_compute(
                                "AllGather", mybir.AluOpType.bypass,
                                replica_groups=[list(range(N_CORES))],
                                ins=[cc_in[blk][:].opt()],
                                outs=[cc_out[blk][:].opt()])
                    if blk >= 1:
                        emit_oproj_mg(ops, osb, wo_sb, blk - 1,
                                      blk_out_r[blk - 1], 0)
    nc.compile()
    return nc


_NC_CACHE = None


def _get_nc():
    global _NC_CACHE
    if _NC_CACHE is None:
        _NC_CACHE = _build()
    return _NC_CACHE


def make_in_maps(hidden_states, position_ids, Wq, Wk, Wv, Wo):
    hs = np.ascontiguousarray(
        np.asarray(hidden_states, dtype=np.float32).reshape(TOK, HID).T
    ).astype(NPBF16)
    pos = np.asarray(position_ids, dtype=np.float32)
    inv = 1.0 / (10000.0 ** (np.arange(0, D, 2, dtype=np.float32) / D))
    fr = pos[:, None] * inv[None, :]                     # [S, 64]
    emb = np.concatenate([fr, fr], axis=-1)              # [S, D]
    cos = np.cos(emb).T.astype(np.float32)               # [D, S]
    sin = np.sin(emb).T.astype(np.float32)
    sin[:64] *= -1.0                                     # fold rotate-half sign
    perm = np.zeros((128, 128), np.float32)
    perm[np.arange(128), (np.arange(128) + 64) % 128] = 1.0
    ident = np.eye(128, dtype=np.float32)
    tri = (np.arange(128)[:, None] <= np.arange(128)[None, :]).astype(np.float32)
    ones4 = np.zeros((128, 128), np.float32)
    ones4[[0, 32, 64, 96], :] = 1.0

    scale = 1.0 / np.sqrt(D)
    Wq = np.asarray(Wq, dtype=np.float32)
    Wk = np.asarray(Wk, dtype=np.float32)
    Wv = np.asarray(Wv, dtype=np.float32)
    Wo = np.asarray(Wo, dtype=np.float32)

    in_maps = []
    for c in range(N_CORES):
        in_maps.append({
            "hid_t": hs,
            "wq_t": np.ascontiguousarray(
                (Wq[c * QO:(c + 1) * QO] * scale).T).astype(NPBF16),
            "wk_t": np.ascontiguousarray(Wk[c * D:(c + 1) * D].T).astype(NPBF16),
            "wv_t": np.ascontiguousarray(Wv[c * D:(c + 1) * D].T).astype(NPBF16),
            "wo_t": np.ascontiguousarray(Wo[c * QO:(c + 1) * QO].T).astype(NPBF16),
            "cos_t": cos,
            "sin_t": sin,
            "perm": perm.astype(NPBF16),
            "ident": ident.astype(NPBF16),
            "tri": tri.astype(NPBF16),
            "ones4": ones4.astype(NPBF16),
        })
    return in_maps


def assemble(results):
    full = np.empty((TOK, HID), np.float32)
    for c in range(N_CORES):
        full[:, c * QO:(c + 1) * QO] = results[c]["out"]
    return full.reshape(B, S, HID)


_RUNNER_CACHE = None


def _make_runner(nc):
    """Build the sharded PJRT callable once so repeat kernel() calls skip
    re-tracing; mirrors concourse.bass2jax.run_bass_via_pjrt."""
    import jax
    from jax.sharding import Mesh, PartitionSpec, NamedSharding
    from jax.experimental.shard_map import shard_map
    from concourse import bass2jax

    bass2jax.install_neuronx_cc_hook()
    partition_name = nc.partition_id_tensor.name if nc.partition_id_tensor else None
    in_names, out_names, out_avals = [], [], []
    for alloc in nc.m.functions[0].allocations:
        if not isinstance(alloc, mybir.MemoryLocationSet):
            continue
        name = alloc.memorylocations[0].name
        if alloc.kind == "ExternalInput":
            if name != partition_name:
                in_names.append(name)
        elif alloc.kind == "ExternalOutput":
            out_names.append(name)
            out_avals.append(jax.core.ShapedArray(
                tuple(alloc.tensor_shape), mybir.dt.np(alloc.dtype)))
    n_params, n_outs = len(in_names), len(out_avals)

    def _body(*args):
        operands = list(args)
        if partition_name is not None:
            operands.append(bass2jax.partition_id_tensor())
        return tuple(bass2jax._bass_exec_p.bind(
            *operands,
            out_avals=tuple(out_avals),
            in_names=tuple(in_names + out_names
                           + ([partition_name] if partition_name else [])),
            out_names=tuple(out_names),
            lowering_input_output_aliases=(),
            sim_require_finite=True,
            sim_require_nnan=True,
            nc=nc,
        ))

    devices = jax.devices()[:N_CORES]
    mesh = Mesh(np.asarray(devices), ("core",))
    fn = jax.jit(
        shard_map(_body, mesh=mesh,
                  in_specs=(PartitionSpec("core"),) * (n_params + n_outs),
                  out_specs=(PartitionSpec("core"),) * n_outs,
                  check_rep=False),
        keep_unused=True,
    )
    sharding = NamedSharding(mesh, PartitionSpec("core"))

    def run(in_maps):
        per_core = [[np.asarray(m[name]) for name in in_names] for m in in_maps]
        concat_in = [
            np.concatenate([per_core[c][i] for c in range(N_CORES)], axis=0)
            for i in range(n_params)
        ]
        concat_zeros = [
            np.zeros((N_CORES * a.shape[0], *a.shape[1:]), a.dtype)
            for a in out_avals
        ]
        import jax as _jax
        dev_args = [_jax.device_put(a, sharding)
                    for a in concat_in + concat_zeros]
        outs = fn(*dev_args)
        _jax.block_until_ready(outs)
        return [
            {name: np.asarray(outs[i]).reshape(N_CORES, *out_avals[i].shape)[c]
             for i, name in enumerate(out_names)}
            for c in range(N_CORES)
        ]

    return run


def kernel(hidden_states, position_ids, Wq, Wk, Wv, Wo):
    global _RUNNER_CACHE
    nc = _get_nc()
    in_maps = make_in_maps(hidden_states, position_ids, Wq, Wk, Wv, Wo)
    try:
        if _RUNNER_CACHE is None:
            _RUNNER_CACHE = _make_runner(nc)
        return assemble(_RUNNER_CACHE(in_maps))
    except Exception:
        from concourse.bass_utils import run_bass_kernel_spmd
        res = run_bass_kernel_spmd(nc, in_maps, core_ids=list(range(N_CORES)))
        return assemble(res.results)

